# revision 2
# baseline (speedup 1.0000x reference)
"""Token-sharded Trainium2 Bass kernel for nn_LogicGatedSpikingSelfAttention.

Sharding: each of the 8 cores owns a 512-token slice (half a batch) and
computes ALL 1024 channels for its tokens. This removes the baseline's
1MB spike AllGather between attention and projection entirely.

Attention is linear (no softmax), so (Q K^T) V is reassociated as
Q (K^T V): per head a [64, 64] integer count matrix KV, turning the
O(N^2) attention into two tiny GEMM passes (~30x fewer FLOPs). All
spike tensors are {0,1} so every attention quantity is integer-exact
(max KV count 7, max S 13, max E 49 on these inputs; bf16 is exact for
integers up to 256).

Collectives (all small):
  - per-branch BN stats: AllGather of the per-512-token-chunk bn_stats
    records (3KB); bn_aggr then sees the exact same [128, 8, 6] input a
    single-core implementation would see -> bitwise-identical thresholds.
  - KV: pair AllReduce (128KB bf16) between the two cores sharing a
    batch; energy: separate 32B pair AllReduce.
  - proj BN stats: AllGather as above.

Numerics match the baseline bitwise: same bf16 input rounding, same
fp32 PSUM accumulation order (contraction tiles ascending), same
bn_stats/bn_aggr pipeline, same gate pipeline (integer energies cast
to bf16, folded Wg in fp32).
"""
import numpy as np
import ml_dtypes

import concourse.bass as bass
import concourse.bacc as bacc
import concourse.tile as tile
from concourse import mybir
from concourse.bass_utils import run_bass_kernel_spmd

NCORES = 8
B, NSEQ, D, H = 4, 1024, 1024, 16
HD = D // H            # 64 head dim
TLOC = (B * NSEQ) // NCORES   # 512 tokens per core
KT = D // 128          # 8 contraction tiles
OT = D // 128          # 8 output-channel tiles
HP = H // 2            # 8 head pairs (= channel tiles)
TT = TLOC // 128       # 4 token tiles
TOK = B * NSEQ
EPS = 1e-5
KVLEN = 128 * HP * HD  # 65536 elements in the KV exchange payload
F32 = mybir.dt.float32
BF16 = mybir.dt.bfloat16
BF = ml_dtypes.bfloat16

_CACHE = {}


def _build():
    nc = bacc.Bacc("TRN2", target_bir_lowering=False, debug=False,
                   num_devices=NCORES)
    inp = {}
    def din(name, shape, dt=BF16):
        inp[name] = nc.dram_tensor(name, shape, dt, kind="ExternalInput")
        return inp[name]

    din("xT", [128, KT * TLOC])              # [p, (kt n)] local-token x^T
    for nm in ("wv", "wk", "wq", "wp"):      # [p, (ot kt co)] lhsT tiles
        din(nm, [128, OT * KT * 128])
    for nm in ("tv", "tk", "tq", "tp", "bv", "bk", "bq", "bp"):
        din(nm, [128, OT], F32)
    din("wgr", [H, H], F32)                  # lhsT: sum_r Wg[h', h+16r]/1024
    din("bgr", [H, 1], F32)
    din("i2e", [128, 2], F32)                # [p, j] = (p//64==j)
    din("maskD", [H, OT], F32)               # [h, o] = (h//2 == o)
    din("indH", [H, 128], F32)               # [h, p] = (p//64 == h%2)
    din("idn", [128, 128])                   # identity for PE transpose
    outT = nc.dram_tensor("outT", [128, OT * TLOC], BF16,
                          kind="ExternalOutput")

    with tile.TileContext(nc) as tc:
        with tc.tile_pool(name="consts", bufs=1) as consts, \
             tc.tile_pool(name="spk", bufs=1) as spk, \
             tc.tile_pool(name="dram", bufs=1, space="DRAM") as dram:
            _body(tc, inp, outT, consts, spk, dram)
    nc.compile()
    return nc


def _body(tc, inp, outT, consts, spk, dram):
    nc = tc.nc
    V, SC, GP, TE, SY = nc.vector, nc.scalar, nc.gpsimd, nc.tensor, nc.sync
    AF = mybir.ActivationFunctionType
    OP = mybir.AluOpType
    ALL = [list(range(NCORES))]
    PAIRS = [[2 * i, 2 * i + 1] for i in range(4)]

    # ---- constants ----
    small = {}
    for nm in ("tv", "tk", "tq", "tp", "bv", "bk", "bq", "bp"):
        t = consts.tile([128, OT], F32, name=nm)
        SY.dma_start(t[:], inp[nm].ap())
        small[nm] = t
    wgr = consts.tile([H, H], F32)
    SY.dma_start(wgr[:], inp["wgr"].ap())
    bgr = consts.tile([H, 1], F32)
    SY.dma_start(bgr[:], inp["bgr"].ap())
    i2e = consts.tile([128, 2], F32)
    SY.dma_start(i2e[:], inp["i2e"].ap())
    maskD = consts.tile([H, OT], F32)
    SY.dma_start(maskD[:], inp["maskD"].ap())
    indH = consts.tile([H, 128], F32)
    SY.dma_start(indH[:], inp["indH"].ap())
    idn = consts.tile([128, 128], BF16)
    SC.dma_start(idn[:], inp["idn"].ap())
    eps = consts.tile([128, 1], F32)
    V.memset(eps[:], EPS)

    # ---- x load (chunked so matmuls can start early) ----
    xts = spk.tile([128, KT, TLOC], BF16, name="xts")
    for kt in range(KT):
        [SC, GP, SY][kt % 3].dma_start(
            xts[:, kt, :], inp["xT"][:, kt * TLOC:(kt + 1) * TLOC])

    # ---- weight stream (per-branch tags, spread across queue groups) ----
    with tc.tile_pool(name="wpool", bufs=6) as wpool:
        wt = {}
        for nm in ("v", "k", "q", "p"):
            wt[nm] = []
            for ot in range(OT):
                t = wpool.tile([128, KT, 128], BF16, tag="w",
                               name=f"w{nm}{ot}")
                src = inp["w" + nm][:, ot * KT * 128:(ot + 1) * KT * 128]
                SY.dma_start(t[:], src.rearrange("p (k c) -> p k c", k=KT))
                wt[nm].append(t)

        # ---- persistent tensors ----
        sp = {nm: spk.tile([128, OT, TLOC], BF16, name=f"sp{nm}")
              for nm in ("v", "k", "q")}
        knat = spk.tile([128, TT, HP, 128], BF16, name="knat")
        vnat = spk.tile([128, TT, HP, 128], BF16, name="vnat")
        kvp = spk.tile([128, HP, HD], BF16, name="kvp")    # self partial
        kvf = spk.tile([128, HP, HD], BF16, name="kvf")    # pair sum
        xat = spk.tile([128, HP, TLOC], BF16, name="xat")
        st = {nm: spk.tile([128, OT, 6], F32, name=f"st{nm}")
              for nm in ("v", "k", "q", "p")}
        stg = {nm: spk.tile([128, 8, OT, 6], F32, name=f"stg{nm}")
               for nm in ("v", "k", "q", "p")}
        mv = {nm: spk.tile([128, OT, 2], F32, name=f"mv{nm}")
              for nm in ("v", "k", "q", "p")}
        std_ = {nm: spk.tile([128, OT], F32, name=f"std{nm}")
                for nm in ("v", "k", "q", "p")}
        thr = {nm: spk.tile([128, OT, 16], F32, name=f"thr{nm}")
               for nm in ("v", "k", "q", "p")}
        prod = spk.tile([128, OT, TLOC], BF16, name="prod")
        Ech = spk.tile([128, OT], F32, name="Ech")
        e_bf = spk.tile([2, OT], BF16, name="e_bf")
        eg_bf = spk.tile([H, 1], BF16, name="eg_bf")
        eg = spk.tile([H, 1], F32, name="eg")
        gate = spk.tile([H, 1], F32, name="gate")
        gateM = spk.tile([H, OT], F32, name="gateM")
        thrG = spk.tile([128, HP, 16], F32, name="thrG")
        osb = spk.tile([128, OT, TLOC], BF16, name="osb")

        # ---- DRAM collective buffers ----
        st_d = {nm: dram.tile([128 * OT * 6], F32, name=f"std_{nm}")
                for nm in ("v", "k", "q", "p")}
        stg_d = {nm: dram.tile([NCORES * 128 * OT * 6], F32,
                               name=f"stgd_{nm}", addr_space="Shared")
                 for nm in ("v", "k", "q", "p")}
        kv_d = dram.tile([KVLEN], BF16, name="kv_d")
        kv2_d = dram.tile([KVLEN], BF16, name="kv2_d")
        e_d = dram.tile([H], BF16, name="e_d")
        e2_d = dram.tile([H], BF16, name="e2_d")

        ypool = [spk.tile([128, OT, TLOC], F32, name=f"Y{i}")
                 for i in range(2)]
        Yb = {"v": ypool[0], "k": ypool[1], "q": ypool[0], "p": ypool[1]}

        def branch_mm(nm, pp):
            Y = Yb[nm]
            for ot in range(OT):
                ps = pp.tile([128, TLOC], F32, tag="ps")
                w = wt[nm][ot]
                for kt in range(KT):
                    TE.matmul(ps[:], w[:, kt, :], xts[:, kt, :],
                              start=(kt == 0), stop=(kt == KT - 1))
                SC.activation(Y[:, ot, :], ps[:], AF.Identity,
                              bias=small["b" + nm][:, ot:ot + 1])
                V.bn_stats(st[nm][:, ot, :], Y[:, ot, :])
            GP.dma_start(
                st_d[nm][:].rearrange("(p o s) -> p o s", p=128, o=OT),
                st[nm][:])
            GP.collective_compute(
                "AllGather", OP.bypass, ins=[st_d[nm].opt()],
                outs=[stg_d[nm].opt()], replica_groups=ALL)
            GP.dma_start(
                stg[nm][:],
                stg_d[nm][:].rearrange("(c p o s) -> p c o s",
                                         c=NCORES, p=128, o=OT))

        def branch_fin(nm):
            # thresholds: bitwise-identical bn_aggr input to single-core
            Y = Yb[nm]
            for o in range(OT):
                V.bn_aggr(mv[nm][:, o, :], stg[nm][:, :, o, :])
            SC.activation(std_[nm][:], mv[nm][:, :, 1:2], AF.Sqrt,
                          bias=eps[:])
            V.tensor_tensor(thr[nm][:, :, 0:1], std_[nm][:],
                            small["t" + nm][:], OP.mult)
            V.tensor_tensor(thr[nm][:, :, 0:1], thr[nm][:, :, 0:1],
                            mv[nm][:, :, 0:1], OP.add)
            dst = sp[nm] if nm != "p" else osb
            for o in range(OT):
                V.tensor_scalar(dst[:, o, :], Y[:, o, :],
                                thr[nm][:, o, 0:1], None, OP.is_ge)

        def transposes(nm, nat, tp_pool):
            for oc in range(HP):
                for t in range(TT):
                    i = oc * TT + t
                    tp = tp_pool.tile([128, 128], BF16, tag="t")
                    TE.transpose(tp[:], sp[nm][:, oc, t * 128:(t + 1) * 128],
                                 idn[:])
                    eng = V if i % 2 else SC
                    if eng is V:
                        V.tensor_copy(nat[:, t, oc, :], tp[:])
                    else:
                        SC.activation(nat[:, t, oc, :], tp[:], AF.Copy)

        # ================= branches =================
        with tc.tile_pool(name="brps", bufs=3, space="PSUM") as brps, \
             tc.tile_pool(name="tps", bufs=4, space="PSUM") as tps:
            branch_mm("v", brps)
            branch_mm("k", brps)
            branch_fin("v")
            transposes("v", vnat, tps)
            branch_mm("q", brps)
            branch_fin("k")
            transposes("k", knat, tps)

        # ================= KV partial + exchange =================
        with tc.tile_pool(name="kvps", bufs=3, space="PSUM") as kvps, \
             tc.tile_pool(name="gps", bufs=2, space="PSUM") as gps:
            for hp in range(HP):
                kvt = kvps.tile([128, HD], F32, tag="kv")
                for h2 in (0, 1):
                    for t in range(TT):
                        TE.matmul(kvt[64 * h2:64 * (h2 + 1), :],
                                  knat[:, t, hp, 64 * h2:64 * (h2 + 1)],
                                  vnat[:, t, hp, 64 * h2:64 * (h2 + 1)],
                                  start=(t == 0), stop=(t == TT - 1))
                SC.activation(kvp[:, hp, :], kvt[:], AF.Copy)
            GP.dma_start(
                kv_d[:].rearrange("(p t d) -> p t d", p=128, t=HP),
                kvp[:])
            GP.collective_compute(
                "AllReduce", OP.add, ins=[kv_d.opt()], outs=[kv2_d.opt()],
                replica_groups=PAIRS)
            SC.dma_start(
                kvf[:],
                kv2_d[:].rearrange("(p t d) -> p t d", p=128, t=HP))

            # ---- q finish + energy (parallel with KV exchange) ----
            branch_fin("q")
            for o in range(OT):
                eng = V if o % 2 else GP
                eng.tensor_tensor(prod[:, o, :], sp["q"][:, o, :],
                                  sp["k"][:, o, :], OP.mult)
            V.reduce_sum(Ech[:], prod[:], axis=mybir.AxisListType.X)
            e_ps = gps.tile([2, OT], F32, tag="g")
            TE.matmul(e_ps[:], i2e[:], Ech[:], start=True, stop=True)
            V.tensor_copy(e_bf[:], e_ps[:])
            GP.dma_start(e_d[:].rearrange("(o j) -> j o", o=OT, j=2),
                         e_bf[:])
            GP.collective_compute(
                "AllReduce", OP.add, ins=[e_d.opt()], outs=[e2_d.opt()],
                replica_groups=PAIRS)
            GP.dma_start(eg_bf[:],
                         e2_d[:].rearrange("(h w) -> h w", w=1))
            V.tensor_copy(eg[:], eg_bf[:])

            # ---- gate -> per-partition threshold vector ----
            g_ps = gps.tile([H, 1], F32, tag="g")
            TE.matmul(g_ps[:], wgr[:], eg[:], start=True, stop=True)
            V.tensor_scalar(gate[:], g_ps[:], bgr[:], 0.5, OP.add, OP.is_ge)
            V.tensor_scalar(gateM[:], maskD[:], gate[:], None, OP.mult)
            g2_ps = gps.tile([128, OT], F32, tag="g2")
            TE.matmul(g2_ps[:], indH[:], gateM[:], start=True, stop=True)
            V.tensor_scalar(thrG[:, :, 0:1], g2_ps[:], -1.0e6, 1.0e6 + 2.0,
                            OP.mult, OP.add)

        # ================= S^T = KV^T q  + proj =================
        with tc.tile_pool(name="sps", bufs=5, space="PSUM") as sps, \
             tc.tile_pool(name="pps", bufs=3, space="PSUM") as pps:
            s_tiles = []
            for hp in range(HP):
                s_ps = sps.tile([128, TLOC], F32, tag="s")
                for h2 in (0, 1):
                    sl = slice(64 * h2, 64 * (h2 + 1))
                    TE.matmul(s_ps[sl, :], kvf[sl, hp, :], sp["q"][sl, hp, :],
                              start=True, stop=True)
                V.tensor_scalar(xat[:, hp, :], s_ps[:],
                                thrG[:, hp, 0:1], None, OP.is_ge)
                s_tiles.append(s_ps)

            for ot in range(OT):
                ps = pps.tile([128, TLOC], F32, tag="pp")
                w = wt["p"][ot]
                for hp in range(HP):
                    TE.matmul(ps[:], w[:, hp, :], xat[:, hp, :],
                              start=(hp == 0), stop=(hp == HP - 1))
                SC.activation(Yb["p"][:, ot, :], ps[:], AF.Identity,
                              bias=small["bp"][:, ot:ot + 1])
                V.bn_stats(st["p"][:, ot, :], Yb["p"][:, ot, :])
            GP.dma_start(
                st_d["p"][:].rearrange("(p o s) -> p o s", p=128, o=OT),
                st["p"][:])
            GP.collective_compute(
                "AllGather", OP.bypass, ins=[st_d["p"].opt()],
                outs=[stg_d["p"].opt()], replica_groups=ALL)
            GP.dma_start(
                stg["p"][:],
                stg_d["p"][:].rearrange("(c p o s) -> p c o s",
                                          c=NCORES, p=128, o=OT))

        # ================= final BN threshold + out =================
        branch_fin("p")
        for o in range(OT):
            [SC, GP, SY][o % 3].dma_start(
                outT.ap()[:, o * TLOC:(o + 1) * TLOC], osb[:, o, :])


def _prep_inputs(inputs):
    x = np.asarray(inputs["x"], np.float32).reshape(TOK, D)
    shared = {}
    for nm in ("q", "k", "v", "p"):
        W = np.asarray(inputs[f"W{nm}"], np.float32)
        A = np.ascontiguousarray(W.T).astype(BF)      # [in, out] lhsT
        shared["w" + nm] = np.ascontiguousarray(
            A.reshape(KT, 128, OT, 128).transpose(1, 2, 0, 3)
        ).reshape(128, OT * KT * 128)
        g = np.asarray(inputs[f"g{nm}"], np.float32)
        be = np.asarray(inputs[f"beta{nm}"], np.float32)
        shared["t" + nm] = np.ascontiguousarray(
            ((2.0 - be) / g).reshape(OT, 128).T)
        shared["b" + nm] = np.ascontiguousarray(
            np.asarray(inputs[f"b{nm}"], np.float32).reshape(OT, 128).T)
    Wg = np.asarray(inputs["Wg"], np.float64)
    wgr = (Wg.reshape(H, HD, H).sum(axis=1).T / 1024.0).astype(np.float32)
    shared["wgr"] = np.ascontiguousarray(wgr)
    shared["bgr"] = np.asarray(inputs["bg"], np.float32).reshape(H, 1)
    i2e = np.zeros((128, 2), np.float32)
    i2e[0:64, 0] = 1.0
    i2e[64:128, 1] = 1.0
    shared["i2e"] = i2e
    maskD = np.zeros((H, OT), np.float32)
    for h in range(H):
        maskD[h, h // 2] = 1.0
    shared["maskD"] = maskD
    indH = np.zeros((H, 128), np.float32)
    for h in range(H):
        base = (h % 2) * 64
        indH[h, base:base + 64] = 1.0
    shared["indH"] = indH
    shared["idn"] = np.eye(128, dtype=BF)

    in_maps = []
    for c in range(NCORES):
        xT = np.ascontiguousarray(x[c * TLOC:(c + 1) * TLOC].T).astype(BF)
        xT = np.ascontiguousarray(
            xT.reshape(KT, 128, TLOC).transpose(1, 0, 2)
        ).reshape(128, KT * TLOC)
        m = dict(shared)
        m["xT"] = xT
        in_maps.append(m)
    return in_maps


def _run(inputs, trace=False):
    if "nc" not in _CACHE:
        _CACHE["nc"] = _build()
    nc = _CACHE["nc"]
    in_maps = _prep_inputs(inputs)
    res = run_bass_kernel_spmd(nc, in_maps, core_ids=list(range(NCORES)),
                               trace=trace)
    out = np.empty((TOK, D), np.float32)
    for c in range(NCORES):
        r = res.results[c]["outT"].astype(np.float32).reshape(128, OT, TLOC)
        out[c * TLOC:(c + 1) * TLOC, :] = \
            r.transpose(2, 1, 0).reshape(TLOC, D)
    return out.reshape(B, NSEQ, D), res


def kernel(**inputs) -> np.ndarray:
    out, _ = _run(inputs, trace=False)
    return out


# revision 3
# speedup vs baseline: 1.0081x; 1.0081x over previous
"""Token-sharded Trainium2 Bass kernel for nn_LogicGatedSpikingSelfAttention.

Sharding: each of the 8 cores owns a 512-token slice (half a batch) and
computes ALL 1024 channels for its tokens. This removes the baseline's
1MB spike AllGather between attention and projection entirely.

Attention is linear (no softmax), so (Q K^T) V is reassociated as
Q (K^T V): per head a [64, 64] integer count matrix KV, turning the
O(N^2) attention into two tiny GEMM passes (~30x fewer FLOPs). All
spike tensors are {0,1} so every attention quantity is integer-exact
(max KV count 7, max S 13, max E 49 on these inputs; bf16 is exact for
integers up to 256).

Collectives (all small):
  - per-branch BN stats: AllGather of the per-512-token-chunk bn_stats
    records (3KB); bn_aggr then sees the exact same [128, 8, 6] input a
    single-core implementation would see -> bitwise-identical thresholds.
  - KV: pair AllReduce (128KB bf16) between the two cores sharing a
    batch; energy: separate 32B pair AllReduce.
  - proj BN stats: AllGather as above.

Numerics match the baseline bitwise: same bf16 input rounding, same
fp32 PSUM accumulation order (contraction tiles ascending), same
bn_stats/bn_aggr pipeline, same gate pipeline (integer energies cast
to bf16, folded Wg in fp32).
"""
import numpy as np
import ml_dtypes

import concourse.bass as bass
import concourse.bacc as bacc
import concourse.tile as tile
from concourse import mybir
from concourse.bass_utils import run_bass_kernel_spmd

NCORES = 8
B, NSEQ, D, H = 4, 1024, 1024, 16
HD = D // H            # 64 head dim
TLOC = (B * NSEQ) // NCORES   # 512 tokens per core
KT = D // 128          # 8 contraction tiles
OT = D // 128          # 8 output-channel tiles
HP = H // 2            # 8 head pairs (= channel tiles)
TT = TLOC // 128       # 4 token tiles
TOK = B * NSEQ
EPS = 1e-5
KVLEN = 128 * HP * HD  # 65536 elements in the KV exchange payload
F32 = mybir.dt.float32
BF16 = mybir.dt.bfloat16
BF = ml_dtypes.bfloat16

_CACHE = {}


def _build():
    nc = bacc.Bacc("TRN2", target_bir_lowering=False, debug=False,
                   num_devices=NCORES)
    inp = {}
    def din(name, shape, dt=BF16):
        inp[name] = nc.dram_tensor(name, shape, dt, kind="ExternalInput")
        return inp[name]

    din("xT", [128, KT * TLOC])              # [p, (kt n)] local-token x^T
    for nm in ("wv", "wk", "wq", "wp"):      # [p, (ot kt co)] lhsT tiles
        din(nm, [128, OT * KT * 128])
    for nm in ("tv", "tk", "tq", "tp", "bv", "bk", "bq", "bp"):
        din(nm, [128, OT], F32)
    din("wgr", [H, H], F32)                  # lhsT: sum_r Wg[h', h+16r]/1024
    din("bgr", [H, 1], F32)
    din("i2e", [128, 2], F32)                # [p, j] = (p//64==j)
    din("maskD", [H, OT], F32)               # [h, o] = (h//2 == o)
    din("indH", [H, 128], F32)               # [h, p] = (p//64 == h%2)
    din("idn", [128, 128])                   # identity for PE transpose
    outT = nc.dram_tensor("outT", [128, OT * TLOC], BF16,
                          kind="ExternalOutput")

    with tile.TileContext(nc) as tc:
        with tc.tile_pool(name="consts", bufs=1) as consts, \
             tc.tile_pool(name="spk", bufs=1) as spk, \
             tc.tile_pool(name="dram", bufs=1, space="DRAM") as dram:
            _body(tc, inp, outT, consts, spk, dram)
    nc.compile()
    return nc


def _body(tc, inp, outT, consts, spk, dram):
    nc = tc.nc
    V, SC, GP, TE, SY = nc.vector, nc.scalar, nc.gpsimd, nc.tensor, nc.sync
    AF = mybir.ActivationFunctionType
    OP = mybir.AluOpType
    ALL = [list(range(NCORES))]
    PAIRS = [[2 * i, 2 * i + 1] for i in range(4)]

    # ---- constants ----
    small = {}
    for nm in ("tv", "tk", "tq", "tp", "bv", "bk", "bq", "bp"):
        t = consts.tile([128, OT], F32, name=nm)
        SY.dma_start(t[:], inp[nm].ap())
        small[nm] = t
    wgr = consts.tile([H, H], F32)
    SY.dma_start(wgr[:], inp["wgr"].ap())
    bgr = consts.tile([H, 1], F32)
    SY.dma_start(bgr[:], inp["bgr"].ap())
    i2e = consts.tile([128, 2], F32)
    SY.dma_start(i2e[:], inp["i2e"].ap())
    maskD = consts.tile([H, OT], F32)
    SY.dma_start(maskD[:], inp["maskD"].ap())
    indH = consts.tile([H, 128], F32)
    SY.dma_start(indH[:], inp["indH"].ap())
    idn = consts.tile([128, 128], BF16)
    SC.dma_start(idn[:], inp["idn"].ap())
    eps = consts.tile([128, 1], F32)
    V.memset(eps[:], EPS)

    # ---- x load (chunked so matmuls can start early) ----
    xts = spk.tile([128, KT, TLOC], BF16, name="xts")
    for kt in range(KT):
        [SC, GP, SY][kt % 3].dma_start(
            xts[:, kt, :], inp["xT"][:, kt * TLOC:(kt + 1) * TLOC])

    # ---- weight stream (per-branch tags, spread across queue groups) ----
    WENG = {"v": SY, "k": SC, "q": SY, "p": SC}
    with tc.tile_pool(name="wpool", bufs=1) as wpool:
        wt = {}
        for nm in ("v", "k", "q", "p"):
            wt[nm] = []
            for ot in range(OT):
                t = wpool.tile([128, KT, 128], BF16, name=f"w{nm}{ot}")
                src = inp["w" + nm][:, ot * KT * 128:(ot + 1) * KT * 128]
                WENG[nm].dma_start(t[:],
                                   src.rearrange("p (k c) -> p k c", k=KT))
                wt[nm].append(t)

        # ---- persistent tensors ----
        sp = {nm: spk.tile([128, OT, TLOC], BF16, name=f"sp{nm}")
              for nm in ("v", "k", "q")}
        knat = spk.tile([128, TT, HP, 128], BF16, name="knat")
        vnat = spk.tile([128, TT, HP, 128], BF16, name="vnat")
        kvp = spk.tile([128, HP, HD], BF16, name="kvp")    # self partial
        kvf = spk.tile([128, HP, HD], BF16, name="kvf")    # pair sum
        xat = spk.tile([128, HP, TLOC], BF16, name="xat")
        st = {nm: spk.tile([128, OT, 6], F32, name=f"st{nm}")
              for nm in ("v", "k", "q", "p")}
        stg = {nm: spk.tile([128, 8, OT, 6], F32, name=f"stg{nm}")
               for nm in ("v", "k", "q", "p")}
        mv = {nm: spk.tile([128, OT, 2], F32, name=f"mv{nm}")
              for nm in ("v", "k", "q", "p")}
        std_ = {nm: spk.tile([128, OT], F32, name=f"std{nm}")
                for nm in ("v", "k", "q", "p")}
        thr = {nm: spk.tile([128, OT, 16], F32, name=f"thr{nm}")
               for nm in ("v", "k", "q", "p")}
        prod = spk.tile([128, OT, TLOC], BF16, name="prod")
        Ech = spk.tile([128, OT], F32, name="Ech")
        e_bf = spk.tile([2, OT], BF16, name="e_bf")
        eg_bf = spk.tile([H, 1], BF16, name="eg_bf")
        eg = spk.tile([H, 1], F32, name="eg")
        gate = spk.tile([H, 1], F32, name="gate")
        gateM = spk.tile([H, OT], F32, name="gateM")
        thrG = spk.tile([128, HP, 16], F32, name="thrG")
        osb = spk.tile([128, OT, TLOC], BF16, name="osb")

        # ---- warm-up collective (absorb CC spin-up latency) ----
        warm_d = dram.tile([16], F32, name="warm_d")
        warmg_d = dram.tile([NCORES * 16], F32, name="warmg_d",
                            addr_space="Shared")
        warm_sb = spk.tile([1, 16], F32, name="warm_sb")
        V.memset(warm_sb[:], 0.0)
        GP.dma_start(warm_d[:].rearrange("(p w) -> p w", p=1), warm_sb[:])
        GP.collective_compute(
            "AllGather", OP.bypass, ins=[warm_d.opt()],
            outs=[warmg_d.opt()], replica_groups=ALL)

        # ---- DRAM collective buffers ----
        st_d = {nm: dram.tile([128 * OT * 6], F32, name=f"std_{nm}")
                for nm in ("v", "k", "q", "p")}
        stg_d = {nm: dram.tile([NCORES * 128 * OT * 6], F32,
                               name=f"stgd_{nm}", addr_space="Shared")
                 for nm in ("v", "k", "q", "p")}
        kv_d = dram.tile([KVLEN], BF16, name="kv_d")
        kv2_d = dram.tile([KVLEN], BF16, name="kv2_d")
        e_d = dram.tile([H], BF16, name="e_d")
        e2_d = dram.tile([H], BF16, name="e2_d")

        ypool = [spk.tile([128, OT, TLOC], F32, name=f"Y{i}")
                 for i in range(2)]
        Yb = {"v": ypool[0], "k": ypool[1], "q": ypool[0], "p": ypool[1]}

        def branch_mm(nm, pp):
            Y = Yb[nm]
            for ot in range(OT):
                ps = pp.tile([128, TLOC], F32, tag="ps")
                w = wt[nm][ot]
                for kt in range(KT):
                    TE.matmul(ps[:], w[:, kt, :], xts[:, kt, :],
                              start=(kt == 0), stop=(kt == KT - 1))
                SC.activation(Y[:, ot, :], ps[:], AF.Identity,
                              bias=small["b" + nm][:, ot:ot + 1])
                V.bn_stats(st[nm][:, ot, :], Y[:, ot, :])
            GP.dma_start(
                st_d[nm][:].rearrange("(p o s) -> p o s", p=128, o=OT),
                st[nm][:])
            GP.collective_compute(
                "AllGather", OP.bypass, ins=[st_d[nm].opt()],
                outs=[stg_d[nm].opt()], replica_groups=ALL)
            GP.dma_start(
                stg[nm][:],
                stg_d[nm][:].rearrange("(c p o s) -> p c o s",
                                         c=NCORES, p=128, o=OT))

        def branch_fin(nm):
            # thresholds: bitwise-identical bn_aggr input to single-core
            Y = Yb[nm]
            for o in range(OT):
                V.bn_aggr(mv[nm][:, o, :], stg[nm][:, :, o, :])
            SC.activation(std_[nm][:], mv[nm][:, :, 1:2], AF.Sqrt,
                          bias=eps[:])
            V.tensor_tensor(thr[nm][:, :, 0:1], std_[nm][:],
                            small["t" + nm][:], OP.mult)
            V.tensor_tensor(thr[nm][:, :, 0:1], thr[nm][:, :, 0:1],
                            mv[nm][:, :, 0:1], OP.add)
            dst = sp[nm] if nm != "p" else osb
            for o in range(OT):
                V.tensor_scalar(dst[:, o, :], Y[:, o, :],
                                thr[nm][:, o, 0:1], None, OP.is_ge)

        def transposes(nm, nat, tp_pool):
            for oc in range(HP):
                for t in range(TT):
                    i = oc * TT + t
                    tp = tp_pool.tile([128, 128], BF16, tag="t")
                    TE.transpose(tp[:], sp[nm][:, oc, t * 128:(t + 1) * 128],
                                 idn[:])
                    eng = V if i % 2 else SC
                    if eng is V:
                        V.tensor_copy(nat[:, t, oc, :], tp[:])
                    else:
                        SC.activation(nat[:, t, oc, :], tp[:], AF.Copy)

        # ================= branches =================
        with tc.tile_pool(name="brps", bufs=3, space="PSUM") as brps, \
             tc.tile_pool(name="tps", bufs=4, space="PSUM") as tps:
            branch_mm("v", brps)
            branch_mm("k", brps)
            branch_fin("v")
            transposes("v", vnat, tps)
            branch_mm("q", brps)
            branch_fin("k")
            transposes("k", knat, tps)

        # ================= KV partial + exchange =================
        with tc.tile_pool(name="kvps", bufs=3, space="PSUM") as kvps, \
             tc.tile_pool(name="gps", bufs=2, space="PSUM") as gps:
            for hp in range(HP):
                kvt = kvps.tile([128, HD], F32, tag="kv")
                for h2 in (0, 1):
                    for t in range(TT):
                        TE.matmul(kvt[64 * h2:64 * (h2 + 1), :],
                                  knat[:, t, hp, 64 * h2:64 * (h2 + 1)],
                                  vnat[:, t, hp, 64 * h2:64 * (h2 + 1)],
                                  start=(t == 0), stop=(t == TT - 1))
                SC.activation(kvp[:, hp, :], kvt[:], AF.Copy)
            GP.dma_start(
                kv_d[:].rearrange("(p t d) -> p t d", p=128, t=HP),
                kvp[:])
            GP.collective_compute(
                "AllReduce", OP.add, ins=[kv_d.opt()], outs=[kv2_d.opt()],
                replica_groups=PAIRS)
            SC.dma_start(
                kvf[:],
                kv2_d[:].rearrange("(p t d) -> p t d", p=128, t=HP))

            # ---- q finish + energy (parallel with KV exchange) ----
            branch_fin("q")
            for o in range(OT):
                eng = V if o % 2 else GP
                eng.tensor_tensor(prod[:, o, :], sp["q"][:, o, :],
                                  sp["k"][:, o, :], OP.mult)
            V.reduce_sum(Ech[:], prod[:], axis=mybir.AxisListType.X)
            e_ps = gps.tile([2, OT], F32, tag="g")
            TE.matmul(e_ps[:], i2e[:], Ech[:], start=True, stop=True)
            V.tensor_copy(e_bf[:], e_ps[:])
            GP.dma_start(e_d[:].rearrange("(o j) -> j o", o=OT, j=2),
                         e_bf[:])
            GP.collective_compute(
                "AllReduce", OP.add, ins=[e_d.opt()], outs=[e2_d.opt()],
                replica_groups=PAIRS)
            GP.dma_start(eg_bf[:],
                         e2_d[:].rearrange("(h w) -> h w", w=1))
            V.tensor_copy(eg[:], eg_bf[:])

            # ---- gate -> per-partition threshold vector ----
            g_ps = gps.tile([H, 1], F32, tag="g")
            TE.matmul(g_ps[:], wgr[:], eg[:], start=True, stop=True)
            V.tensor_scalar(gate[:], g_ps[:], bgr[:], 0.5, OP.add, OP.is_ge)
            V.tensor_scalar(gateM[:], maskD[:], gate[:], None, OP.mult)
            g2_ps = gps.tile([128, OT], F32, tag="g2")
            TE.matmul(g2_ps[:], indH[:], gateM[:], start=True, stop=True)
            V.tensor_scalar(thrG[:, :, 0:1], g2_ps[:], -1.0e6, 1.0e6 + 2.0,
                            OP.mult, OP.add)

        # ================= S^T = KV^T q  + proj =================
        with tc.tile_pool(name="sps", bufs=5, space="PSUM") as sps, \
             tc.tile_pool(name="pps", bufs=3, space="PSUM") as pps:
            s_tiles = []
            for hp in range(HP):
                s_ps = sps.tile([128, TLOC], F32, tag="s")
                for h2 in (0, 1):
                    sl = slice(64 * h2, 64 * (h2 + 1))
                    TE.matmul(s_ps[sl, :], kvf[sl, hp, :], sp["q"][sl, hp, :],
                              start=True, stop=True)
                V.tensor_scalar(xat[:, hp, :], s_ps[:],
                                thrG[:, hp, 0:1], None, OP.is_ge)
                s_tiles.append(s_ps)

            for ot in range(OT):
                ps = pps.tile([128, TLOC], F32, tag="pp")
                w = wt["p"][ot]
                for hp in range(HP):
                    TE.matmul(ps[:], w[:, hp, :], xat[:, hp, :],
                              start=(hp == 0), stop=(hp == HP - 1))
                SC.activation(Yb["p"][:, ot, :], ps[:], AF.Identity,
                              bias=small["bp"][:, ot:ot + 1])
                V.bn_stats(st["p"][:, ot, :], Yb["p"][:, ot, :])
            GP.dma_start(
                st_d["p"][:].rearrange("(p o s) -> p o s", p=128, o=OT),
                st["p"][:])
            GP.collective_compute(
                "AllGather", OP.bypass, ins=[st_d["p"].opt()],
                outs=[stg_d["p"].opt()], replica_groups=ALL)
            GP.dma_start(
                stg["p"][:],
                stg_d["p"][:].rearrange("(c p o s) -> p c o s",
                                          c=NCORES, p=128, o=OT))

        # ================= final BN threshold + out =================
        branch_fin("p")
        SC.dma_start(outT.ap().rearrange("p (o n) -> p o n", o=OT), osb[:])


def _prep_inputs(inputs):
    x = np.asarray(inputs["x"], np.float32).reshape(TOK, D)
    shared = {}
    for nm in ("q", "k", "v", "p"):
        W = np.asarray(inputs[f"W{nm}"], np.float32)
        A = np.ascontiguousarray(W.T).astype(BF)      # [in, out] lhsT
        shared["w" + nm] = np.ascontiguousarray(
            A.reshape(KT, 128, OT, 128).transpose(1, 2, 0, 3)
        ).reshape(128, OT * KT * 128)
        g = np.asarray(inputs[f"g{nm}"], np.float32)
        be = np.asarray(inputs[f"beta{nm}"], np.float32)
        shared["t" + nm] = np.ascontiguousarray(
            ((2.0 - be) / g).reshape(OT, 128).T)
        shared["b" + nm] = np.ascontiguousarray(
            np.asarray(inputs[f"b{nm}"], np.float32).reshape(OT, 128).T)
    Wg = np.asarray(inputs["Wg"], np.float64)
    wgr = (Wg.reshape(H, HD, H).sum(axis=1).T / 1024.0).astype(np.float32)
    shared["wgr"] = np.ascontiguousarray(wgr)
    shared["bgr"] = np.asarray(inputs["bg"], np.float32).reshape(H, 1)
    i2e = np.zeros((128, 2), np.float32)
    i2e[0:64, 0] = 1.0
    i2e[64:128, 1] = 1.0
    shared["i2e"] = i2e
    maskD = np.zeros((H, OT), np.float32)
    for h in range(H):
        maskD[h, h // 2] = 1.0
    shared["maskD"] = maskD
    indH = np.zeros((H, 128), np.float32)
    for h in range(H):
        base = (h % 2) * 64
        indH[h, base:base + 64] = 1.0
    shared["indH"] = indH
    shared["idn"] = np.eye(128, dtype=BF)

    in_maps = []
    for c in range(NCORES):
        xT = np.ascontiguousarray(x[c * TLOC:(c + 1) * TLOC].T).astype(BF)
        xT = np.ascontiguousarray(
            xT.reshape(KT, 128, TLOC).transpose(1, 0, 2)
        ).reshape(128, KT * TLOC)
        m = dict(shared)
        m["xT"] = xT
        in_maps.append(m)
    return in_maps


def _run(inputs, trace=False):
    if "nc" not in _CACHE:
        _CACHE["nc"] = _build()
    nc = _CACHE["nc"]
    in_maps = _prep_inputs(inputs)
    res = run_bass_kernel_spmd(nc, in_maps, core_ids=list(range(NCORES)),
                               trace=trace)
    out = np.empty((TOK, D), np.float32)
    for c in range(NCORES):
        r = res.results[c]["outT"].astype(np.float32).reshape(128, OT, TLOC)
        out[c * TLOC:(c + 1) * TLOC, :] = \
            r.transpose(2, 1, 0).reshape(TLOC, D)
    return out.reshape(B, NSEQ, D), res


def kernel(**inputs) -> np.ndarray:
    out, _ = _run(inputs, trace=False)
    return out


# revision 4
# speedup vs baseline: 1.0422x; 1.0339x over previous
"""Token-sharded Trainium2 Bass kernel for nn_LogicGatedSpikingSelfAttention.

Sharding: each of the 8 cores owns a 512-token slice (half a batch) and
computes ALL 1024 channels for its tokens. This removes the baseline's
1MB spike AllGather between attention and projection entirely.

Attention is linear (no softmax), so (Q K^T) V is reassociated as
Q (K^T V): per head a [64, 64] integer count matrix KV, turning the
O(N^2) attention into two tiny GEMM passes (~30x fewer FLOPs). All
spike tensors are {0,1} so every attention quantity is integer-exact
(max KV count 7, max S 13, max E 49 on these inputs; bf16 is exact for
integers up to 256).

Collectives (all small):
  - per-branch BN stats: AllGather of the per-512-token-chunk bn_stats
    records (3KB); bn_aggr then sees the exact same [128, 8, 6] input a
    single-core implementation would see -> bitwise-identical thresholds.
  - KV: pair AllReduce (128KB bf16) between the two cores sharing a
    batch; energy: separate 32B pair AllReduce.
  - proj BN stats: AllGather as above.

Numerics match the baseline bitwise: same bf16 input rounding, same
fp32 PSUM accumulation order (contraction tiles ascending), same
bn_stats/bn_aggr pipeline, same gate pipeline (integer energies cast
to bf16, folded Wg in fp32).
"""
import numpy as np
import ml_dtypes

import concourse.bass as bass
import concourse.bacc as bacc
import concourse.tile as tile
from concourse import mybir
from concourse.bass_utils import run_bass_kernel_spmd

NCORES = 8
B, NSEQ, D, H = 4, 1024, 1024, 16
HD = D // H            # 64 head dim
TLOC = (B * NSEQ) // NCORES   # 512 tokens per core
KT = D // 128          # 8 contraction tiles
OT = D // 128          # 8 output-channel tiles
HP = H // 2            # 8 head pairs (= channel tiles)
TT = TLOC // 128       # 4 token tiles
TOK = B * NSEQ
EPS = 1e-5
KVLEN = 128 * HP * HD  # 65536 elements in the KV exchange payload
F32 = mybir.dt.float32
BF16 = mybir.dt.bfloat16
BF = ml_dtypes.bfloat16

_CACHE = {}


def _build():
    nc = bacc.Bacc("TRN2", target_bir_lowering=False, debug=False,
                   num_devices=NCORES)
    inp = {}
    def din(name, shape, dt=BF16):
        inp[name] = nc.dram_tensor(name, shape, dt, kind="ExternalInput")
        return inp[name]

    din("xT", [128, KT * TLOC])              # [p, (kt n)] local-token x^T
    for nm in ("wv", "wk", "wq", "wp"):      # [p, (ot kt co)] lhsT tiles
        din(nm, [128, OT * KT * 128])
    for nm in ("tv", "tk", "tq", "tp", "bv", "bk", "bq", "bp"):
        din(nm, [128, OT], F32)
    din("wgr", [H, H], F32)                  # lhsT: sum_r Wg[h', h+16r]/1024
    din("bgr", [H, 1], F32)
    din("i2e", [128, 2], F32)                # [p, j] = (p//64==j)
    din("maskD", [H, OT], F32)               # [h, o] = (h//2 == o)
    din("indH", [H, 128], F32)               # [h, p] = (p//64 == h%2)
    din("idn", [128, 128])                   # identity for PE transpose
    outT = nc.dram_tensor("outT", [128, OT * TLOC], BF16,
                          kind="ExternalOutput")

    with tile.TileContext(nc) as tc:
        with tc.tile_pool(name="consts", bufs=1) as consts, \
             tc.tile_pool(name="spk", bufs=1) as spk, \
             tc.tile_pool(name="dram", bufs=1, space="DRAM") as dram:
            _body(tc, inp, outT, consts, spk, dram)
    nc.compile()
    return nc


def _body(tc, inp, outT, consts, spk, dram):
    nc = tc.nc
    V, SC, GP, TE, SY = nc.vector, nc.scalar, nc.gpsimd, nc.tensor, nc.sync
    AF = mybir.ActivationFunctionType
    OP = mybir.AluOpType
    ALL = [list(range(NCORES))]
    PAIRS = [[2 * i, 2 * i + 1] for i in range(4)]

    # ---- constants ----
    small = {}
    for nm in ("tv", "tk", "tq", "tp", "bv", "bk", "bq", "bp"):
        t = consts.tile([128, OT], F32, name=nm)
        SY.dma_start(t[:], inp[nm].ap())
        small[nm] = t
    wgr = consts.tile([H, H], F32)
    SY.dma_start(wgr[:], inp["wgr"].ap())
    bgr = consts.tile([H, 1], F32)
    SY.dma_start(bgr[:], inp["bgr"].ap())
    i2e = consts.tile([128, 2], F32)
    SY.dma_start(i2e[:], inp["i2e"].ap())
    maskD = consts.tile([H, OT], F32)
    SY.dma_start(maskD[:], inp["maskD"].ap())
    indH = consts.tile([H, 128], F32)
    SY.dma_start(indH[:], inp["indH"].ap())
    idn = consts.tile([128, 128], BF16)
    SC.dma_start(idn[:], inp["idn"].ap())
    eps = consts.tile([128, 1], F32)
    V.memset(eps[:], EPS)

    # ---- x load (one DMA: 8KB contiguous per partition) ----
    xts = spk.tile([128, KT, TLOC], BF16, name="xts")
    GP.dma_start(xts[:],
                 inp["xT"].ap().rearrange("p (k n) -> p k n", k=KT))

    # ---- weight stream (per-branch tags, spread across queue groups) ----
    WENG = {"v": SY, "k": SC, "q": SY, "p": SC}
    with tc.tile_pool(name="wpool", bufs=1) as wpool:
        wt = {}
        for nm in ("v", "k", "q", "p"):
            wt[nm] = []
            for ot in range(OT):
                t = wpool.tile([128, KT, 128], BF16, name=f"w{nm}{ot}")
                src = inp["w" + nm][:, ot * KT * 128:(ot + 1) * KT * 128]
                WENG[nm].dma_start(t[:],
                                   src.rearrange("p (k c) -> p k c", k=KT))
                wt[nm].append(t)

        # ---- persistent tensors ----
        sp = {nm: spk.tile([128, OT, TLOC], BF16, name=f"sp{nm}")
              for nm in ("v", "k", "q")}
        knat = spk.tile([128, TT, HP, 128], BF16, name="knat")
        vnat = spk.tile([128, TT, HP, 128], BF16, name="vnat")
        kvp = spk.tile([128, HP, HD], BF16, name="kvp")    # self partial
        kvf = spk.tile([128, HP, HD], BF16, name="kvf")    # pair sum
        xat = spk.tile([128, HP, TLOC], BF16, name="xat")
        st = {nm: spk.tile([128, OT, 6], F32, name=f"st{nm}")
              for nm in ("v", "k", "q", "p")}
        stg = {nm: spk.tile([128, 8, OT, 6], F32, name=f"stg{nm}")
               for nm in ("v", "k", "q", "p")}
        mv = {nm: spk.tile([128, OT, 2], F32, name=f"mv{nm}")
              for nm in ("v", "k", "q", "p")}
        std_ = {nm: spk.tile([128, OT], F32, name=f"std{nm}")
                for nm in ("v", "k", "q", "p")}
        thr = {nm: spk.tile([128, OT, 16], F32, name=f"thr{nm}")
               for nm in ("v", "k", "q", "p")}
        prod = spk.tile([128, OT, TLOC], BF16, name="prod")
        Ech = spk.tile([128, OT], F32, name="Ech")
        e_bf = spk.tile([2, OT], BF16, name="e_bf")
        eg_bf = spk.tile([H, 1], BF16, name="eg_bf")
        eg = spk.tile([H, 1], F32, name="eg")
        gate = spk.tile([H, 1], F32, name="gate")
        gateM = spk.tile([H, OT], F32, name="gateM")
        thrG = spk.tile([128, HP, 16], F32, name="thrG")
        osb = spk.tile([128, OT, TLOC], BF16, name="osb")

        # ---- warm-up collective (absorb CC spin-up latency) ----
        warm_d = dram.tile([16], F32, name="warm_d")
        warmg_d = dram.tile([NCORES * 16], F32, name="warmg_d",
                            addr_space="Shared")
        warm_sb = spk.tile([1, 16], F32, name="warm_sb")
        V.memset(warm_sb[:], 0.0)
        GP.dma_start(warm_d[:].rearrange("(p w) -> p w", p=1), warm_sb[:])
        GP.collective_compute(
            "AllGather", OP.bypass, ins=[warm_d.opt()],
            outs=[warmg_d.opt()], replica_groups=ALL)

        # ---- DRAM collective buffers ----
        st_d = {nm: dram.tile([128 * OT * 6], F32, name=f"std_{nm}")
                for nm in ("v", "k", "q", "p")}
        stg_d = {nm: dram.tile([NCORES * 128 * OT * 6], F32,
                               name=f"stgd_{nm}", addr_space="Shared")
                 for nm in ("v", "k", "q", "p")}
        kv_d = dram.tile([KVLEN], BF16, name="kv_d")
        kv2_d = dram.tile([KVLEN], BF16, name="kv2_d")
        e_d = dram.tile([H], BF16, name="e_d")
        e2_d = dram.tile([H], BF16, name="e2_d")

        ypool = [spk.tile([128, OT, TLOC], F32, name=f"Y{i}")
                 for i in range(2)]
        Yb = {"v": ypool[0], "k": ypool[1], "q": ypool[0], "p": ypool[1]}

        def branch_mm(nm, pp):
            Y = Yb[nm]
            for ot in range(OT):
                ps = pp.tile([128, TLOC], F32, tag="ps")
                w = wt[nm][ot]
                for kt in range(KT):
                    TE.matmul(ps[:], w[:, kt, :], xts[:, kt, :],
                              start=(kt == 0), stop=(kt == KT - 1))
                SC.activation(Y[:, ot, :], ps[:], AF.Identity,
                              bias=small["b" + nm][:, ot:ot + 1])
                V.bn_stats(st[nm][:, ot, :], Y[:, ot, :])
            GP.dma_start(
                st_d[nm][:].rearrange("(p o s) -> p o s", p=128, o=OT),
                st[nm][:])
            GP.collective_compute(
                "AllGather", OP.bypass, ins=[st_d[nm].opt()],
                outs=[stg_d[nm].opt()], replica_groups=ALL)
            GP.dma_start(
                stg[nm][:],
                stg_d[nm][:].rearrange("(c p o s) -> p c o s",
                                         c=NCORES, p=128, o=OT))

        def branch_fin(nm):
            # thresholds: bitwise-identical bn_aggr input to single-core
            Y = Yb[nm]
            for o in range(OT):
                V.bn_aggr(mv[nm][:, o, :], stg[nm][:, :, o, :])
            SC.activation(std_[nm][:], mv[nm][:, :, 1:2], AF.Sqrt,
                          bias=eps[:])
            V.tensor_tensor(thr[nm][:, :, 0:1], std_[nm][:],
                            small["t" + nm][:], OP.mult)
            V.tensor_tensor(thr[nm][:, :, 0:1], thr[nm][:, :, 0:1],
                            mv[nm][:, :, 0:1], OP.add)
            dst = sp[nm] if nm != "p" else osb
            for o in range(OT):
                V.tensor_scalar(dst[:, o, :], Y[:, o, :],
                                thr[nm][:, o, 0:1], None, OP.is_ge)

        def transposes(nm, nat, tp_pool):
            for oc in range(HP):
                for t in range(TT):
                    i = oc * TT + t
                    tp = tp_pool.tile([128, 128], BF16, tag="t")
                    TE.transpose(tp[:], sp[nm][:, oc, t * 128:(t + 1) * 128],
                                 idn[:])
                    eng = V if i % 2 else SC
                    if eng is V:
                        V.tensor_copy(nat[:, t, oc, :], tp[:])
                    else:
                        SC.activation(nat[:, t, oc, :], tp[:], AF.Copy)

        # ================= branches =================
        with tc.tile_pool(name="brps", bufs=3, space="PSUM") as brps, \
             tc.tile_pool(name="tps", bufs=4, space="PSUM") as tps:
            branch_mm("v", brps)
            branch_mm("k", brps)
            branch_fin("v")
            transposes("v", vnat, tps)
            branch_mm("q", brps)
            branch_fin("k")
            transposes("k", knat, tps)

        # ================= KV partial + exchange =================
        with tc.tile_pool(name="kvps", bufs=3, space="PSUM") as kvps, \
             tc.tile_pool(name="gps", bufs=2, space="PSUM") as gps:
            for hp in range(HP):
                kvt = kvps.tile([128, HD], F32, tag="kv")
                for h2 in (0, 1):
                    for t in range(TT):
                        TE.matmul(kvt[64 * h2:64 * (h2 + 1), :],
                                  knat[:, t, hp, 64 * h2:64 * (h2 + 1)],
                                  vnat[:, t, hp, 64 * h2:64 * (h2 + 1)],
                                  start=(t == 0), stop=(t == TT - 1))
                SC.activation(kvp[:, hp, :], kvt[:], AF.Copy)
            GP.dma_start(
                kv_d[:].rearrange("(p t d) -> p t d", p=128, t=HP),
                kvp[:])
            GP.collective_compute(
                "AllReduce", OP.add, ins=[kv_d.opt()], outs=[kv2_d.opt()],
                replica_groups=PAIRS)
            SC.dma_start(
                kvf[:],
                kv2_d[:].rearrange("(p t d) -> p t d", p=128, t=HP))

            # ---- q finish + energy (parallel with KV exchange) ----
            branch_fin("q")
            for o in range(OT):
                eng = V if o % 2 else GP
                eng.tensor_tensor(prod[:, o, :], sp["q"][:, o, :],
                                  sp["k"][:, o, :], OP.mult)
            V.reduce_sum(Ech[:], prod[:], axis=mybir.AxisListType.X)
            e_ps = gps.tile([2, OT], F32, tag="g")
            TE.matmul(e_ps[:], i2e[:], Ech[:], start=True, stop=True)
            V.tensor_copy(e_bf[:], e_ps[:])
            GP.dma_start(e_d[:].rearrange("(o j) -> j o", o=OT, j=2),
                         e_bf[:])
            GP.collective_compute(
                "AllReduce", OP.add, ins=[e_d.opt()], outs=[e2_d.opt()],
                replica_groups=PAIRS)
            GP.dma_start(eg_bf[:],
                         e2_d[:].rearrange("(h w) -> h w", w=1))
            V.tensor_copy(eg[:], eg_bf[:])

            # ---- gate -> per-partition threshold vector ----
            g_ps = gps.tile([H, 1], F32, tag="g")
            TE.matmul(g_ps[:], wgr[:], eg[:], start=True, stop=True)
            V.tensor_scalar(gate[:], g_ps[:], bgr[:], 0.5, OP.add, OP.is_ge)
            V.tensor_scalar(gateM[:], maskD[:], gate[:], None, OP.mult)
            g2_ps = gps.tile([128, OT], F32, tag="g2")
            TE.matmul(g2_ps[:], indH[:], gateM[:], start=True, stop=True)
            V.tensor_scalar(thrG[:, :, 0:1], g2_ps[:], -1.0e6, 1.0e6 + 2.0,
                            OP.mult, OP.add)

        # ================= S^T = KV^T q  + proj =================
        with tc.tile_pool(name="sps", bufs=5, space="PSUM") as sps, \
             tc.tile_pool(name="pps", bufs=3, space="PSUM") as pps:
            s_tiles = []
            for hp in range(HP):
                s_ps = sps.tile([128, TLOC], F32, tag="s")
                for h2 in (0, 1):
                    sl = slice(64 * h2, 64 * (h2 + 1))
                    TE.matmul(s_ps[sl, :], kvf[sl, hp, :], sp["q"][sl, hp, :],
                              start=True, stop=True)
                V.tensor_scalar(xat[:, hp, :], s_ps[:],
                                thrG[:, hp, 0:1], None, OP.is_ge)
                s_tiles.append(s_ps)

            for ot in range(OT):
                ps = pps.tile([128, TLOC], F32, tag="pp")
                w = wt["p"][ot]
                for hp in range(HP):
                    TE.matmul(ps[:], w[:, hp, :], xat[:, hp, :],
                              start=(hp == 0), stop=(hp == HP - 1))
                SC.activation(Yb["p"][:, ot, :], ps[:], AF.Identity,
                              bias=small["bp"][:, ot:ot + 1])
                V.bn_stats(st["p"][:, ot, :], Yb["p"][:, ot, :])
            GP.dma_start(
                st_d["p"][:].rearrange("(p o s) -> p o s", p=128, o=OT),
                st["p"][:])
            GP.collective_compute(
                "AllGather", OP.bypass, ins=[st_d["p"].opt()],
                outs=[stg_d["p"].opt()], replica_groups=ALL)
            GP.dma_start(
                stg["p"][:],
                stg_d["p"][:].rearrange("(c p o s) -> p c o s",
                                          c=NCORES, p=128, o=OT))

        # ================= final BN threshold + out =================
        branch_fin("p")
        SC.dma_start(outT.ap().rearrange("p (o n) -> p o n", o=OT), osb[:])


def _prep_inputs(inputs):
    x = np.asarray(inputs["x"], np.float32).reshape(TOK, D)
    shared = {}
    for nm in ("q", "k", "v", "p"):
        W = np.asarray(inputs[f"W{nm}"], np.float32)
        A = np.ascontiguousarray(W.T).astype(BF)      # [in, out] lhsT
        shared["w" + nm] = np.ascontiguousarray(
            A.reshape(KT, 128, OT, 128).transpose(1, 2, 0, 3)
        ).reshape(128, OT * KT * 128)
        g = np.asarray(inputs[f"g{nm}"], np.float32)
        be = np.asarray(inputs[f"beta{nm}"], np.float32)
        shared["t" + nm] = np.ascontiguousarray(
            ((2.0 - be) / g).reshape(OT, 128).T)
        shared["b" + nm] = np.ascontiguousarray(
            np.asarray(inputs[f"b{nm}"], np.float32).reshape(OT, 128).T)
    Wg = np.asarray(inputs["Wg"], np.float64)
    wgr = (Wg.reshape(H, HD, H).sum(axis=1).T / 1024.0).astype(np.float32)
    shared["wgr"] = np.ascontiguousarray(wgr)
    shared["bgr"] = np.asarray(inputs["bg"], np.float32).reshape(H, 1)
    i2e = np.zeros((128, 2), np.float32)
    i2e[0:64, 0] = 1.0
    i2e[64:128, 1] = 1.0
    shared["i2e"] = i2e
    maskD = np.zeros((H, OT), np.float32)
    for h in range(H):
        maskD[h, h // 2] = 1.0
    shared["maskD"] = maskD
    indH = np.zeros((H, 128), np.float32)
    for h in range(H):
        base = (h % 2) * 64
        indH[h, base:base + 64] = 1.0
    shared["indH"] = indH
    shared["idn"] = np.eye(128, dtype=BF)

    in_maps = []
    for c in range(NCORES):
        xT = np.ascontiguousarray(x[c * TLOC:(c + 1) * TLOC].T).astype(BF)
        xT = np.ascontiguousarray(
            xT.reshape(KT, 128, TLOC).transpose(1, 0, 2)
        ).reshape(128, KT * TLOC)
        m = dict(shared)
        m["xT"] = xT
        in_maps.append(m)
    return in_maps


def _run(inputs, trace=False):
    if "nc" not in _CACHE:
        _CACHE["nc"] = _build()
    nc = _CACHE["nc"]
    in_maps = _prep_inputs(inputs)
    res = run_bass_kernel_spmd(nc, in_maps, core_ids=list(range(NCORES)),
                               trace=trace)
    out = np.empty((TOK, D), np.float32)
    for c in range(NCORES):
        r = res.results[c]["outT"].astype(np.float32).reshape(128, OT, TLOC)
        out[c * TLOC:(c + 1) * TLOC, :] = \
            r.transpose(2, 1, 0).reshape(TLOC, D)
    return out.reshape(B, NSEQ, D), res


def kernel(**inputs) -> np.ndarray:
    out, _ = _run(inputs, trace=False)
    return out


# revision 5
# speedup vs baseline: 1.0701x; 1.0268x over previous
"""Token-sharded Trainium2 Bass kernel for nn_LogicGatedSpikingSelfAttention.

Sharding: each of the 8 cores owns a 512-token slice (half a batch) and
computes ALL 1024 channels for its tokens. This removes the baseline's
1MB spike AllGather between attention and projection entirely.

Attention is linear (no softmax), so (Q K^T) V is reassociated as
Q (K^T V): per head a [64, 64] integer count matrix KV, turning the
O(N^2) attention into two tiny GEMM passes (~30x fewer FLOPs). All
spike tensors are {0,1} so every attention quantity is integer-exact
(max KV count 7, max S 13, max E 49 on these inputs; bf16 is exact for
integers up to 256).

Collectives (all small):
  - per-branch BN stats: AllGather of the per-512-token-chunk bn_stats
    records (3KB); bn_aggr then sees the exact same [128, 8, 6] input a
    single-core implementation would see -> bitwise-identical thresholds.
  - KV: pair AllReduce (128KB bf16) between the two cores sharing a
    batch; energy: separate 32B pair AllReduce.
  - proj BN stats: AllGather as above.

Numerics match the baseline bitwise: same bf16 input rounding, same
fp32 PSUM accumulation order (contraction tiles ascending), same
bn_stats/bn_aggr pipeline, same gate pipeline (integer energies cast
to bf16, folded Wg in fp32).
"""
import numpy as np
import ml_dtypes

import concourse.bass as bass
import concourse.bacc as bacc
import concourse.tile as tile
from concourse import mybir
from concourse.bass_utils import run_bass_kernel_spmd

NCORES = 8
B, NSEQ, D, H = 4, 1024, 1024, 16
HD = D // H            # 64 head dim
TLOC = (B * NSEQ) // NCORES   # 512 tokens per core
KT = D // 128          # 8 contraction tiles
OT = D // 128          # 8 output-channel tiles
HP = H // 2            # 8 head pairs (= channel tiles)
TT = TLOC // 128       # 4 token tiles
TOK = B * NSEQ
EPS = 1e-5
KVLEN = 128 * HP * HD  # 65536 elements in the KV exchange payload
F32 = mybir.dt.float32
BF16 = mybir.dt.bfloat16
BF = ml_dtypes.bfloat16

_CACHE = {}


def _build():
    nc = bacc.Bacc("TRN2", target_bir_lowering=False, debug=False,
                   num_devices=NCORES)
    inp = {}
    def din(name, shape, dt=BF16):
        inp[name] = nc.dram_tensor(name, shape, dt, kind="ExternalInput")
        return inp[name]

    din("xT", [128, KT * TLOC])              # [p, (kt n)] local-token x^T
    for nm in ("wv", "wk", "wq", "wp"):      # [p, (ot kt co)] lhsT tiles
        din(nm, [128, OT * KT * 128])
    for nm in ("tv", "tk", "tq", "tp", "bv", "bk", "bq", "bp"):
        din(nm, [128, OT], F32)
    din("wgr", [H, H], F32)                  # lhsT: sum_r Wg[h', h+16r]/1024
    din("bgr", [H, 1], F32)
    din("i2e", [128, 2], F32)                # [p, j] = (p//64==j)
    din("maskD", [H, OT], F32)               # [h, o] = (h//2 == o)
    din("indH", [H, 128], F32)               # [h, p] = (p//64 == h%2)
    din("idn", [128, 128])                   # identity for PE transpose
    outT = nc.dram_tensor("outT", [128, OT * TLOC], BF16,
                          kind="ExternalOutput")

    with tile.TileContext(nc) as tc:
        with tc.tile_pool(name="consts", bufs=1) as consts, \
             tc.tile_pool(name="spk", bufs=1) as spk, \
             tc.tile_pool(name="dram", bufs=1, space="DRAM") as dram:
            _body(tc, inp, outT, consts, spk, dram)
    nc.compile()
    return nc


def _body(tc, inp, outT, consts, spk, dram):
    nc = tc.nc
    V, SC, GP, TE, SY = nc.vector, nc.scalar, nc.gpsimd, nc.tensor, nc.sync
    AF = mybir.ActivationFunctionType
    OP = mybir.AluOpType
    ALL = [list(range(NCORES))]
    PAIRS = [[2 * i, 2 * i + 1] for i in range(4)]

    # ---- constants ----
    small = {}
    for nm in ("tv", "tk", "tq", "tp", "bv", "bk", "bq", "bp"):
        t = consts.tile([128, OT], F32, name=nm)
        SY.dma_start(t[:], inp[nm].ap())
        small[nm] = t
    wgr = consts.tile([H, H], F32)
    SY.dma_start(wgr[:], inp["wgr"].ap())
    bgr = consts.tile([H, 1], F32)
    SY.dma_start(bgr[:], inp["bgr"].ap())
    i2e = consts.tile([128, 2], F32)
    SY.dma_start(i2e[:], inp["i2e"].ap())
    maskD = consts.tile([H, OT], F32)
    SY.dma_start(maskD[:], inp["maskD"].ap())
    indH = consts.tile([H, 128], F32)
    SY.dma_start(indH[:], inp["indH"].ap())
    idn = consts.tile([128, 128], BF16)
    SC.dma_start(idn[:], inp["idn"].ap())
    eps = consts.tile([128, 1], F32)
    V.memset(eps[:], EPS)

    # ---- x load (one DMA: 8KB contiguous per partition) ----
    xts = spk.tile([128, KT, TLOC], BF16, name="xts")
    GP.dma_start(xts[:],
                 inp["xT"].ap().rearrange("p (k n) -> p k n", k=KT))

    # ---- weight stream (per-branch tags, spread across queue groups) ----
    WENG = {"v": GP, "k": SC, "q": SY, "p": SC}
    with tc.tile_pool(name="wpool", bufs=1) as wpool:
        wt = {}
        for nm in ("v", "k", "q", "p"):
            wt[nm] = []
            for ot in range(OT):
                t = wpool.tile([128, KT, 128], BF16, name=f"w{nm}{ot}")
                src = inp["w" + nm][:, ot * KT * 128:(ot + 1) * KT * 128]
                WENG[nm].dma_start(t[:],
                                   src.rearrange("p (k c) -> p k c", k=KT))
                wt[nm].append(t)

        # ---- persistent tensors ----
        sp = {nm: spk.tile([128, OT, TLOC], BF16, name=f"sp{nm}")
              for nm in ("v", "k", "q")}
        knat = spk.tile([128, TT, HP, 128], BF16, name="knat")
        vnat = spk.tile([128, TT, HP, 128], BF16, name="vnat")
        kvp = spk.tile([128, HP, HD], BF16, name="kvp")    # self partial
        kvf = spk.tile([128, HP, HD], BF16, name="kvf")    # pair sum
        xat = spk.tile([128, HP, TLOC], BF16, name="xat")
        st = {nm: spk.tile([128, OT, 6], F32, name=f"st{nm}")
              for nm in ("v", "k", "q", "p")}
        stg = {nm: spk.tile([128, 8, OT, 6], F32, name=f"stg{nm}")
               for nm in ("v", "k", "q", "p")}
        mv = {nm: spk.tile([128, OT, 2], F32, name=f"mv{nm}")
              for nm in ("v", "k", "q", "p")}
        std_ = {nm: spk.tile([128, OT], F32, name=f"std{nm}")
                for nm in ("v", "k", "q", "p")}
        thr = {nm: spk.tile([128, OT, 16], F32, name=f"thr{nm}")
               for nm in ("v", "k", "q", "p")}
        prod = spk.tile([128, OT, TLOC], BF16, name="prod")
        Ech = spk.tile([128, OT], F32, name="Ech")
        e_bf = spk.tile([2, OT], BF16, name="e_bf")
        eg_bf = spk.tile([H, 1], BF16, name="eg_bf")
        eg = spk.tile([H, 1], F32, name="eg")
        gate = spk.tile([H, 1], F32, name="gate")
        gateM = spk.tile([H, OT], F32, name="gateM")
        thrG = spk.tile([128, HP, 16], F32, name="thrG")
        osb = spk.tile([128, OT, TLOC], BF16, name="osb")

        # ---- warm-up collective (absorb CC spin-up latency) ----
        warm_d = dram.tile([16], F32, name="warm_d")
        warmg_d = dram.tile([NCORES * 16], F32, name="warmg_d",
                            addr_space="Shared")
        warm_sb = spk.tile([1, 16], F32, name="warm_sb")
        V.memset(warm_sb[:], 0.0)
        GP.dma_start(warm_d[:].rearrange("(p w) -> p w", p=1), warm_sb[:])
        GP.collective_compute(
            "AllGather", OP.bypass, ins=[warm_d.opt()],
            outs=[warmg_d.opt()], replica_groups=ALL)

        # ---- DRAM collective buffers ----
        st_d = {nm: dram.tile([128 * OT * 6], F32, name=f"std_{nm}")
                for nm in ("v", "k", "q", "p")}
        stg_d = {nm: dram.tile([NCORES * 128 * OT * 6], F32,
                               name=f"stgd_{nm}", addr_space="Shared")
                 for nm in ("v", "k", "q", "p")}
        kv_d = dram.tile([KVLEN], BF16, name="kv_d")
        kv2_d = dram.tile([KVLEN], BF16, name="kv2_d")
        e_d = dram.tile([H], BF16, name="e_d")
        e2_d = dram.tile([H], BF16, name="e2_d")

        ypool = [spk.tile([128, OT, TLOC], F32, name=f"Y{i}")
                 for i in range(3)]
        Yb = {"v": ypool[0], "k": ypool[1], "q": ypool[2], "p": ypool[0]}

        def branch_mm(nm, pp):
            Y = Yb[nm]
            for ot in range(OT):
                ps = pp.tile([128, TLOC], F32, tag="ps")
                w = wt[nm][ot]
                for kt in range(KT):
                    TE.matmul(ps[:], w[:, kt, :], xts[:, kt, :],
                              start=(kt == 0), stop=(kt == KT - 1))
                SC.activation(Y[:, ot, :], ps[:], AF.Identity,
                              bias=small["b" + nm][:, ot:ot + 1])
                V.bn_stats(st[nm][:, ot, :], Y[:, ot, :])
            GP.dma_start(
                st_d[nm][:].rearrange("(p o s) -> p o s", p=128, o=OT),
                st[nm][:])
            GP.collective_compute(
                "AllGather", OP.bypass, ins=[st_d[nm].opt()],
                outs=[stg_d[nm].opt()], replica_groups=ALL)
            GP.dma_start(
                stg[nm][:],
                stg_d[nm][:].rearrange("(c p o s) -> p c o s",
                                         c=NCORES, p=128, o=OT))

        def branch_fin(nm):
            # thresholds: bitwise-identical bn_aggr input to single-core
            Y = Yb[nm]
            for o in range(OT):
                V.bn_aggr(mv[nm][:, o, :], stg[nm][:, :, o, :])
            SC.activation(std_[nm][:], mv[nm][:, :, 1:2], AF.Sqrt,
                          bias=eps[:])
            V.tensor_tensor(thr[nm][:, :, 0:1], std_[nm][:],
                            small["t" + nm][:], OP.mult)
            V.tensor_tensor(thr[nm][:, :, 0:1], thr[nm][:, :, 0:1],
                            mv[nm][:, :, 0:1], OP.add)
            dst = sp[nm] if nm != "p" else osb
            for o in range(OT):
                V.tensor_scalar(dst[:, o, :], Y[:, o, :],
                                thr[nm][:, o, 0:1], None, OP.is_ge)

        def transposes(nm, nat, tp_pool):
            for oc in range(HP):
                for t in range(TT):
                    i = oc * TT + t
                    tp = tp_pool.tile([128, 128], BF16, tag="t")
                    TE.transpose(tp[:], sp[nm][:, oc, t * 128:(t + 1) * 128],
                                 idn[:])
                    eng = V if i % 2 else SC
                    if eng is V:
                        V.tensor_copy(nat[:, t, oc, :], tp[:])
                    else:
                        SC.activation(nat[:, t, oc, :], tp[:], AF.Copy)

        # ================= branches =================
        with tc.tile_pool(name="brps", bufs=3, space="PSUM") as brps, \
             tc.tile_pool(name="tps", bufs=4, space="PSUM") as tps:
            branch_mm("v", brps)
            branch_mm("k", brps)
            branch_mm("q", brps)
            branch_fin("v")
            transposes("v", vnat, tps)
            branch_fin("k")
            transposes("k", knat, tps)

        # ================= KV partial + exchange =================
        with tc.tile_pool(name="kvps", bufs=3, space="PSUM") as kvps, \
             tc.tile_pool(name="gps", bufs=2, space="PSUM") as gps:
            for hp in range(HP):
                kvt = kvps.tile([128, HD], F32, tag="kv")
                for h2 in (0, 1):
                    for t in range(TT):
                        TE.matmul(kvt[64 * h2:64 * (h2 + 1), :],
                                  knat[:, t, hp, 64 * h2:64 * (h2 + 1)],
                                  vnat[:, t, hp, 64 * h2:64 * (h2 + 1)],
                                  start=(t == 0), stop=(t == TT - 1))
                SC.activation(kvp[:, hp, :], kvt[:], AF.Copy)
            GP.dma_start(
                kv_d[:].rearrange("(p t d) -> p t d", p=128, t=HP),
                kvp[:])
            GP.collective_compute(
                "AllReduce", OP.add, ins=[kv_d.opt()], outs=[kv2_d.opt()],
                replica_groups=PAIRS)
            SC.dma_start(
                kvf[:],
                kv2_d[:].rearrange("(p t d) -> p t d", p=128, t=HP))

            # ---- q finish + energy (parallel with KV exchange) ----
            branch_fin("q")
            for o in range(OT):
                eng = V if o % 2 else GP
                eng.tensor_tensor(prod[:, o, :], sp["q"][:, o, :],
                                  sp["k"][:, o, :], OP.mult)
            V.reduce_sum(Ech[:], prod[:], axis=mybir.AxisListType.X)
            e_ps = gps.tile([2, OT], F32, tag="g")
            TE.matmul(e_ps[:], i2e[:], Ech[:], start=True, stop=True)
            V.tensor_copy(e_bf[:], e_ps[:])
            GP.dma_start(e_d[:].rearrange("(o j) -> j o", o=OT, j=2),
                         e_bf[:])
            GP.collective_compute(
                "AllReduce", OP.add, ins=[e_d.opt()], outs=[e2_d.opt()],
                replica_groups=PAIRS)
            GP.dma_start(eg_bf[:],
                         e2_d[:].rearrange("(h w) -> h w", w=1))
            V.tensor_copy(eg[:], eg_bf[:])

            # ---- gate -> per-partition threshold vector ----
            g_ps = gps.tile([H, 1], F32, tag="g")
            TE.matmul(g_ps[:], wgr[:], eg[:], start=True, stop=True)
            V.tensor_scalar(gate[:], g_ps[:], bgr[:], 0.5, OP.add, OP.is_ge)
            V.tensor_scalar(gateM[:], maskD[:], gate[:], None, OP.mult)
            g2_ps = gps.tile([128, OT], F32, tag="g2")
            TE.matmul(g2_ps[:], indH[:], gateM[:], start=True, stop=True)
            V.tensor_scalar(thrG[:, :, 0:1], g2_ps[:], -1.0e6, 1.0e6 + 2.0,
                            OP.mult, OP.add)

        # ================= S^T = KV^T q  + proj =================
        with tc.tile_pool(name="sps", bufs=5, space="PSUM") as sps, \
             tc.tile_pool(name="pps", bufs=3, space="PSUM") as pps:
            s_tiles = []
            for hp in range(HP):
                s_ps = sps.tile([128, TLOC], F32, tag="s")
                for h2 in (0, 1):
                    sl = slice(64 * h2, 64 * (h2 + 1))
                    TE.matmul(s_ps[sl, :], kvf[sl, hp, :], sp["q"][sl, hp, :],
                              start=True, stop=True)
                V.tensor_scalar(xat[:, hp, :], s_ps[:],
                                thrG[:, hp, 0:1], None, OP.is_ge)
                s_tiles.append(s_ps)

            for ot in range(OT):
                ps = pps.tile([128, TLOC], F32, tag="pp")
                w = wt["p"][ot]
                for hp in range(HP):
                    TE.matmul(ps[:], w[:, hp, :], xat[:, hp, :],
                              start=(hp == 0), stop=(hp == HP - 1))
                SC.activation(Yb["p"][:, ot, :], ps[:], AF.Identity,
                              bias=small["bp"][:, ot:ot + 1])
                V.bn_stats(st["p"][:, ot, :], Yb["p"][:, ot, :])
            GP.dma_start(
                st_d["p"][:].rearrange("(p o s) -> p o s", p=128, o=OT),
                st["p"][:])
            GP.collective_compute(
                "AllGather", OP.bypass, ins=[st_d["p"].opt()],
                outs=[stg_d["p"].opt()], replica_groups=ALL)
            GP.dma_start(
                stg["p"][:],
                stg_d["p"][:].rearrange("(c p o s) -> p c o s",
                                          c=NCORES, p=128, o=OT))

        # ================= final BN threshold + out =================
        branch_fin("p")
        SC.dma_start(outT.ap().rearrange("p (o n) -> p o n", o=OT), osb[:])


def _prep_inputs(inputs):
    x = np.asarray(inputs["x"], np.float32).reshape(TOK, D)
    shared = {}
    for nm in ("q", "k", "v", "p"):
        W = np.asarray(inputs[f"W{nm}"], np.float32)
        A = np.ascontiguousarray(W.T).astype(BF)      # [in, out] lhsT
        shared["w" + nm] = np.ascontiguousarray(
            A.reshape(KT, 128, OT, 128).transpose(1, 2, 0, 3)
        ).reshape(128, OT * KT * 128)
        g = np.asarray(inputs[f"g{nm}"], np.float32)
        be = np.asarray(inputs[f"beta{nm}"], np.float32)
        shared["t" + nm] = np.ascontiguousarray(
            ((2.0 - be) / g).reshape(OT, 128).T)
        shared["b" + nm] = np.ascontiguousarray(
            np.asarray(inputs[f"b{nm}"], np.float32).reshape(OT, 128).T)
    Wg = np.asarray(inputs["Wg"], np.float64)
    wgr = (Wg.reshape(H, HD, H).sum(axis=1).T / 1024.0).astype(np.float32)
    shared["wgr"] = np.ascontiguousarray(wgr)
    shared["bgr"] = np.asarray(inputs["bg"], np.float32).reshape(H, 1)
    i2e = np.zeros((128, 2), np.float32)
    i2e[0:64, 0] = 1.0
    i2e[64:128, 1] = 1.0
    shared["i2e"] = i2e
    maskD = np.zeros((H, OT), np.float32)
    for h in range(H):
        maskD[h, h // 2] = 1.0
    shared["maskD"] = maskD
    indH = np.zeros((H, 128), np.float32)
    for h in range(H):
        base = (h % 2) * 64
        indH[h, base:base + 64] = 1.0
    shared["indH"] = indH
    shared["idn"] = np.eye(128, dtype=BF)

    in_maps = []
    for c in range(NCORES):
        xT = np.ascontiguousarray(x[c * TLOC:(c + 1) * TLOC].T).astype(BF)
        xT = np.ascontiguousarray(
            xT.reshape(KT, 128, TLOC).transpose(1, 0, 2)
        ).reshape(128, KT * TLOC)
        m = dict(shared)
        m["xT"] = xT
        in_maps.append(m)
    return in_maps


def _run(inputs, trace=False):
    if "nc" not in _CACHE:
        _CACHE["nc"] = _build()
    nc = _CACHE["nc"]
    in_maps = _prep_inputs(inputs)
    res = run_bass_kernel_spmd(nc, in_maps, core_ids=list(range(NCORES)),
                               trace=trace)
    out = np.empty((TOK, D), np.float32)
    for c in range(NCORES):
        r = res.results[c]["outT"].astype(np.float32).reshape(128, OT, TLOC)
        out[c * TLOC:(c + 1) * TLOC, :] = \
            r.transpose(2, 1, 0).reshape(TLOC, D)
    return out.reshape(B, NSEQ, D), res


def kernel(**inputs) -> np.ndarray:
    out, _ = _run(inputs, trace=False)
    return out


# revision 6
# speedup vs baseline: 1.1217x; 1.0482x over previous
"""Token-sharded Trainium2 Bass kernel for nn_LogicGatedSpikingSelfAttention.

Sharding: each of the 8 cores owns a 512-token slice (half a batch) and
computes ALL 1024 channels for its tokens. This removes the baseline's
1MB spike AllGather between attention and projection entirely.

Attention is linear (no softmax), so (Q K^T) V is reassociated as
Q (K^T V): per head a [64, 64] integer count matrix KV, turning the
O(N^2) attention into two tiny GEMM passes (~30x fewer FLOPs). All
spike tensors are {0,1} so every attention quantity is integer-exact
(max KV count 7, max S 13, max E 49 on these inputs; bf16 is exact for
integers up to 256).

Collectives (all small):
  - per-branch BN stats: AllGather of the per-512-token-chunk bn_stats
    records (3KB); bn_aggr then sees the exact same [128, 8, 6] input a
    single-core implementation would see -> bitwise-identical thresholds.
  - KV: pair AllReduce (128KB bf16) between the two cores sharing a
    batch; energy: separate 32B pair AllReduce.
  - proj BN stats: AllGather as above.

Numerics match the baseline bitwise: same bf16 input rounding, same
fp32 PSUM accumulation order (contraction tiles ascending), same
bn_stats/bn_aggr pipeline, same gate pipeline (integer energies cast
to bf16, folded Wg in fp32).
"""
import numpy as np
import ml_dtypes

import concourse.bass as bass
import concourse.bacc as bacc
import concourse.tile as tile
from concourse import mybir
from concourse.bass_utils import run_bass_kernel_spmd

NCORES = 8
B, NSEQ, D, H = 4, 1024, 1024, 16
HD = D // H            # 64 head dim
TLOC = (B * NSEQ) // NCORES   # 512 tokens per core
KT = D // 128          # 8 contraction tiles
OT = D // 128          # 8 output-channel tiles
HP = H // 2            # 8 head pairs (= channel tiles)
TT = TLOC // 128       # 4 token tiles
TOK = B * NSEQ
EPS = 1e-5
KVLEN = 128 * HP * HD  # 65536 elements in the KV exchange payload
F32 = mybir.dt.float32
BF16 = mybir.dt.bfloat16
BF = ml_dtypes.bfloat16

_CACHE = {}


def _build():
    nc = bacc.Bacc("TRN2", target_bir_lowering=False, debug=False,
                   num_devices=NCORES)
    inp = {}
    def din(name, shape, dt=BF16):
        inp[name] = nc.dram_tensor(name, shape, dt, kind="ExternalInput")
        return inp[name]

    din("xT", [128, KT * TLOC])              # [p, (kt n)] local-token x^T
    for nm in ("wv", "wk", "wq", "wp"):      # [p, (ot kt co)] lhsT tiles
        din(nm, [128, OT * KT * 128])
    for nm in ("tv", "tk", "tq", "tp", "bv", "bk", "bq", "bp"):
        din(nm, [128, OT], F32)
    din("wgr", [H, H], F32)                  # lhsT: sum_r Wg[h', h+16r]/1024
    din("bgr", [H, 1], F32)
    din("i2e", [128, 2], F32)                # [p, j] = (p//64==j)
    din("maskD", [H, OT], F32)               # [h, o] = (h//2 == o)
    din("indH", [H, 128], F32)               # [h, p] = (p//64 == h%2)
    din("idn", [128, 128])                   # identity for PE transpose
    outT = nc.dram_tensor("outT", [128, OT * TLOC], BF16,
                          kind="ExternalOutput")

    with tile.TileContext(nc) as tc:
        with tc.tile_pool(name="consts", bufs=1) as consts, \
             tc.tile_pool(name="spk", bufs=1) as spk, \
             tc.tile_pool(name="dram", bufs=1, space="DRAM") as dram:
            _body(tc, inp, outT, consts, spk, dram)
    nc.compile()
    return nc


def _body(tc, inp, outT, consts, spk, dram):
    nc = tc.nc
    V, SC, GP, TE, SY = nc.vector, nc.scalar, nc.gpsimd, nc.tensor, nc.sync
    AF = mybir.ActivationFunctionType
    OP = mybir.AluOpType
    ALL = [list(range(NCORES))]
    PAIRS = [[2 * i, 2 * i + 1] for i in range(4)]

    # ---- constants ----
    small = {}
    for nm in ("tv", "tk", "tq", "tp", "bv", "bk", "bq", "bp"):
        t = consts.tile([128, OT], F32, name=nm)
        SY.dma_start(t[:], inp[nm].ap())
        small[nm] = t
    wgr = consts.tile([H, H], F32)
    SY.dma_start(wgr[:], inp["wgr"].ap())
    bgr = consts.tile([H, 1], F32)
    SY.dma_start(bgr[:], inp["bgr"].ap())
    i2e = consts.tile([128, 2], F32)
    SY.dma_start(i2e[:], inp["i2e"].ap())
    maskD = consts.tile([H, OT], F32)
    SY.dma_start(maskD[:], inp["maskD"].ap())
    indH = consts.tile([H, 128], F32)
    SY.dma_start(indH[:], inp["indH"].ap())
    idn = consts.tile([128, 128], BF16)
    SC.dma_start(idn[:], inp["idn"].ap())
    eps = consts.tile([128, 1], F32)
    V.memset(eps[:], EPS)

    # ---- x load (one DMA: 8KB contiguous per partition) ----
    xts = spk.tile([128, KT, TLOC], BF16, name="xts")
    GP.dma_start(xts[:],
                 inp["xT"].ap().rearrange("p (k n) -> p k n", k=KT))

    # ---- weight stream (per-branch tags, spread across queue groups) ----
    WENG = {"v": GP, "k": SC, "q": SY, "p": SC}
    with tc.tile_pool(name="wpool", bufs=1) as wpool:
        wt = {}
        for nm in ("v", "k", "q", "p"):
            wt[nm] = []
            for ot in range(OT):
                t = wpool.tile([128, KT, 128], BF16, name=f"w{nm}{ot}")
                src = inp["w" + nm][:, ot * KT * 128:(ot + 1) * KT * 128]
                WENG[nm].dma_start(t[:],
                                   src.rearrange("p (k c) -> p k c", k=KT))
                wt[nm].append(t)

        # ---- persistent tensors ----
        sp = {nm: spk.tile([128, OT, TLOC], BF16, name=f"sp{nm}")
              for nm in ("v", "k", "q")}
        knat = spk.tile([128, TT, HP, 128], BF16, name="knat")
        vnat = spk.tile([128, TT, HP, 128], BF16, name="vnat")
        kvp = spk.tile([128, HP, HD], BF16, name="kvp")    # self partial
        kvf = spk.tile([128, HP, HD], BF16, name="kvf")    # pair sum
        xat = spk.tile([128, HP, TLOC], BF16, name="xat")
        st = {nm: spk.tile([128, OT, 6], F32, name=f"st{nm}")
              for nm in ("v", "k", "q", "p")}
        stg = {nm: spk.tile([128, 8, OT, 6], F32, name=f"stg{nm}")
               for nm in ("v", "k", "q", "p")}
        mv = {nm: spk.tile([128, OT, 2], F32, name=f"mv{nm}")
              for nm in ("v", "k", "q", "p")}
        std_ = {nm: spk.tile([128, OT], F32, name=f"std{nm}")
                for nm in ("v", "k", "q", "p")}
        thr = {nm: spk.tile([128, OT, 16], F32, name=f"thr{nm}")
               for nm in ("v", "k", "q", "p")}
        prod = spk.tile([128, OT, TLOC], BF16, name="prod")
        Ech = spk.tile([128, OT], F32, name="Ech")
        e_bf = spk.tile([2, OT], BF16, name="e_bf")
        eg_bf = spk.tile([H, 1], BF16, name="eg_bf")
        eg = spk.tile([H, 1], F32, name="eg")
        gate = spk.tile([H, 1], F32, name="gate")
        gateM = spk.tile([H, OT], F32, name="gateM")
        thrG = spk.tile([128, HP, 16], F32, name="thrG")
        osb = spk.tile([128, OT, TLOC], BF16, name="osb")

        # ---- warm-up collective (absorb CC spin-up latency) ----
        warm_d = dram.tile([16], F32, name="warm_d")
        warmg_d = dram.tile([NCORES * 16], F32, name="warmg_d",
                            addr_space="Shared")
        warm_sb = spk.tile([1, 16], F32, name="warm_sb")
        V.memset(warm_sb[:], 0.0)
        GP.dma_start(warm_d[:].rearrange("(p w) -> p w", p=1), warm_sb[:])
        GP.collective_compute(
            "AllGather", OP.bypass, ins=[warm_d.opt()],
            outs=[warmg_d.opt()], replica_groups=ALL)

        # ---- DRAM collective buffers ----
        st_d = {nm: dram.tile([128 * OT * 6], F32, name=f"std_{nm}")
                for nm in ("v", "k", "q", "p")}
        stg_d = {nm: dram.tile([NCORES * 128 * OT * 6], F32,
                               name=f"stgd_{nm}", addr_space="Shared")
                 for nm in ("v", "k", "q", "p")}
        kv_d = dram.tile([KVLEN], BF16, name="kv_d")
        kv2_d = dram.tile([KVLEN], BF16, name="kv2_d")
        e_d = dram.tile([H], BF16, name="e_d")
        e2_d = dram.tile([H], BF16, name="e2_d")
        stp_a = dram.tile([128 * 4 * 6], F32, name="stp_a")
        stpg_a = dram.tile([NCORES * 128 * 4 * 6], F32, name="stpg_a",
                           addr_space="Shared")
        stp_b = dram.tile([128 * 4 * 6], F32, name="stp_b")
        stpg_b = dram.tile([NCORES * 128 * 4 * 6], F32, name="stpg_b",
                           addr_space="Shared")

        ypool = [spk.tile([128, OT, TLOC], F32, name=f"Y{i}")
                 for i in range(3)]
        Yb = {"v": ypool[0], "k": ypool[1], "q": ypool[2], "p": ypool[0]}

        def branch_mm(nm, pp):
            Y = Yb[nm]
            for ot in range(OT):
                ps = pp.tile([128, TLOC], F32, tag="ps")
                w = wt[nm][ot]
                for kt in range(KT):
                    TE.matmul(ps[:], w[:, kt, :], xts[:, kt, :],
                              start=(kt == 0), stop=(kt == KT - 1))
                SC.activation(Y[:, ot, :], ps[:], AF.Identity,
                              bias=small["b" + nm][:, ot:ot + 1])
                V.bn_stats(st[nm][:, ot, :], Y[:, ot, :])
            GP.dma_start(
                st_d[nm][:].rearrange("(p o s) -> p o s", p=128, o=OT),
                st[nm][:])
            GP.collective_compute(
                "AllGather", OP.bypass, ins=[st_d[nm].opt()],
                outs=[stg_d[nm].opt()], replica_groups=ALL)
            GP.dma_start(
                stg[nm][:],
                stg_d[nm][:].rearrange("(c p o s) -> p c o s",
                                         c=NCORES, p=128, o=OT))

        def branch_fin(nm):
            # thresholds: bitwise-identical bn_aggr input to single-core
            Y = Yb[nm]
            for o in range(OT):
                V.bn_aggr(mv[nm][:, o, :], stg[nm][:, :, o, :])
            SC.activation(std_[nm][:], mv[nm][:, :, 1:2], AF.Sqrt,
                          bias=eps[:])
            V.tensor_tensor(thr[nm][:, :, 0:1], std_[nm][:],
                            small["t" + nm][:], OP.mult)
            V.tensor_tensor(thr[nm][:, :, 0:1], thr[nm][:, :, 0:1],
                            mv[nm][:, :, 0:1], OP.add)
            dst = sp[nm] if nm != "p" else osb
            for o in range(OT):
                V.tensor_scalar(dst[:, o, :], Y[:, o, :],
                                thr[nm][:, o, 0:1], None, OP.is_ge)

        def transposes(nm, nat, tp_pool):
            for oc in range(HP):
                for t in range(TT):
                    i = oc * TT + t
                    tp = tp_pool.tile([128, 128], BF16, tag="t")
                    TE.transpose(tp[:], sp[nm][:, oc, t * 128:(t + 1) * 128],
                                 idn[:])
                    eng = V if i % 2 else SC
                    if eng is V:
                        V.tensor_copy(nat[:, t, oc, :], tp[:])
                    else:
                        SC.activation(nat[:, t, oc, :], tp[:], AF.Copy)

        # ================= branches =================
        with tc.tile_pool(name="brps", bufs=3, space="PSUM") as brps, \
             tc.tile_pool(name="tps", bufs=4, space="PSUM") as tps:
            branch_mm("v", brps)
            branch_mm("k", brps)
            branch_mm("q", brps)
            branch_fin("v")
            transposes("v", vnat, tps)
            branch_fin("k")
            transposes("k", knat, tps)

        # ================= KV partial + exchange =================
        with tc.tile_pool(name="kvps", bufs=3, space="PSUM") as kvps, \
             tc.tile_pool(name="gps", bufs=2, space="PSUM") as gps:
            for hp in range(HP):
                kvt = kvps.tile([128, HD], F32, tag="kv")
                for h2 in (0, 1):
                    for t in range(TT):
                        TE.matmul(kvt[64 * h2:64 * (h2 + 1), :],
                                  knat[:, t, hp, 64 * h2:64 * (h2 + 1)],
                                  vnat[:, t, hp, 64 * h2:64 * (h2 + 1)],
                                  start=(t == 0), stop=(t == TT - 1))
                SC.activation(kvp[:, hp, :], kvt[:], AF.Copy)
            GP.dma_start(
                kv_d[:].rearrange("(p t d) -> p t d", p=128, t=HP),
                kvp[:])
            GP.collective_compute(
                "AllReduce", OP.add, ins=[kv_d.opt()], outs=[kv2_d.opt()],
                replica_groups=PAIRS)
            SC.dma_start(
                kvf[:],
                kv2_d[:].rearrange("(p t d) -> p t d", p=128, t=HP))

            # ---- q finish + energy (parallel with KV exchange) ----
            branch_fin("q")
            for o in range(OT):
                eng = V if o % 2 else GP
                eng.tensor_tensor(prod[:, o, :], sp["q"][:, o, :],
                                  sp["k"][:, o, :], OP.mult)
            V.reduce_sum(Ech[:], prod[:], axis=mybir.AxisListType.X)
            e_ps = gps.tile([2, OT], F32, tag="g")
            TE.matmul(e_ps[:], i2e[:], Ech[:], start=True, stop=True)
            V.tensor_copy(e_bf[:], e_ps[:])
            GP.dma_start(e_d[:].rearrange("(o j) -> j o", o=OT, j=2),
                         e_bf[:])
            GP.collective_compute(
                "AllReduce", OP.add, ins=[e_d.opt()], outs=[e2_d.opt()],
                replica_groups=PAIRS)
            GP.dma_start(eg_bf[:],
                         e2_d[:].rearrange("(h w) -> h w", w=1))
            V.tensor_copy(eg[:], eg_bf[:])

            # ---- gate -> per-partition threshold vector ----
            g_ps = gps.tile([H, 1], F32, tag="g")
            TE.matmul(g_ps[:], wgr[:], eg[:], start=True, stop=True)
            V.tensor_scalar(gate[:], g_ps[:], bgr[:], 0.5, OP.add, OP.is_ge)
            V.tensor_scalar(gateM[:], maskD[:], gate[:], None, OP.mult)
            g2_ps = gps.tile([128, OT], F32, tag="g2")
            TE.matmul(g2_ps[:], indH[:], gateM[:], start=True, stop=True)
            V.tensor_scalar(thrG[:, :, 0:1], g2_ps[:], -1.0e6, 1.0e6 + 2.0,
                            OP.mult, OP.add)

        # ================= S^T = KV^T q  + proj =================
        with tc.tile_pool(name="sps", bufs=5, space="PSUM") as sps, \
             tc.tile_pool(name="pps", bufs=3, space="PSUM") as pps:
            s_tiles = []
            for hp in range(HP):
                s_ps = sps.tile([128, TLOC], F32, tag="s")
                for h2 in (0, 1):
                    sl = slice(64 * h2, 64 * (h2 + 1))
                    TE.matmul(s_ps[sl, :], kvf[sl, hp, :], sp["q"][sl, hp, :],
                              start=True, stop=True)
                V.tensor_scalar(xat[:, hp, :], s_ps[:],
                                thrG[:, hp, 0:1], None, OP.is_ge)
                s_tiles.append(s_ps)

            for ot in range(OT):
                ps = pps.tile([128, TLOC], F32, tag="pp")
                w = wt["p"][ot]
                for hp in range(HP):
                    TE.matmul(ps[:], w[:, hp, :], xat[:, hp, :],
                              start=(hp == 0), stop=(hp == HP - 1))
                SC.activation(Yb["p"][:, ot, :], ps[:], AF.Identity,
                              bias=small["bp"][:, ot:ot + 1])
                V.bn_stats(st["p"][:, ot, :], Yb["p"][:, ot, :])
                if ot == 3:
                    GP.dma_start(
                        stp_a[:].rearrange("(p o s) -> p o s",
                                           p=128, o=4),
                        st["p"][:, 0:4, :])
                    GP.collective_compute(
                        "AllGather", OP.bypass, ins=[stp_a.opt()],
                        outs=[stpg_a.opt()], replica_groups=ALL)
                    GP.dma_start(
                        stg["p"][:, :, 0:4, :],
                        stpg_a[:].rearrange("(c p o s) -> p c o s",
                                            c=NCORES, p=128, o=4))
            GP.dma_start(
                stp_b[:].rearrange("(p o s) -> p o s", p=128, o=4),
                st["p"][:, 4:8, :])
            GP.collective_compute(
                "AllGather", OP.bypass, ins=[stp_b.opt()],
                outs=[stpg_b.opt()], replica_groups=ALL)
            GP.dma_start(
                stg["p"][:, :, 4:8, :],
                stpg_b[:].rearrange("(c p o s) -> p c o s",
                                    c=NCORES, p=128, o=4))

        # ================= final BN threshold + out =================
        branch_fin("p")
        SC.dma_start(outT.ap().rearrange("p (o n) -> p o n", o=OT), osb[:])


def _prep_inputs(inputs):
    x = np.asarray(inputs["x"], np.float32).reshape(TOK, D)
    shared = {}
    for nm in ("q", "k", "v", "p"):
        W = np.asarray(inputs[f"W{nm}"], np.float32)
        A = np.ascontiguousarray(W.T).astype(BF)      # [in, out] lhsT
        shared["w" + nm] = np.ascontiguousarray(
            A.reshape(KT, 128, OT, 128).transpose(1, 2, 0, 3)
        ).reshape(128, OT * KT * 128)
        g = np.asarray(inputs[f"g{nm}"], np.float32)
        be = np.asarray(inputs[f"beta{nm}"], np.float32)
        shared["t" + nm] = np.ascontiguousarray(
            ((2.0 - be) / g).reshape(OT, 128).T)
        shared["b" + nm] = np.ascontiguousarray(
            np.asarray(inputs[f"b{nm}"], np.float32).reshape(OT, 128).T)
    Wg = np.asarray(inputs["Wg"], np.float64)
    wgr = (Wg.reshape(H, HD, H).sum(axis=1).T / 1024.0).astype(np.float32)
    shared["wgr"] = np.ascontiguousarray(wgr)
    shared["bgr"] = np.asarray(inputs["bg"], np.float32).reshape(H, 1)
    i2e = np.zeros((128, 2), np.float32)
    i2e[0:64, 0] = 1.0
    i2e[64:128, 1] = 1.0
    shared["i2e"] = i2e
    maskD = np.zeros((H, OT), np.float32)
    for h in range(H):
        maskD[h, h // 2] = 1.0
    shared["maskD"] = maskD
    indH = np.zeros((H, 128), np.float32)
    for h in range(H):
        base = (h % 2) * 64
        indH[h, base:base + 64] = 1.0
    shared["indH"] = indH
    shared["idn"] = np.eye(128, dtype=BF)

    in_maps = []
    for c in range(NCORES):
        xT = np.ascontiguousarray(x[c * TLOC:(c + 1) * TLOC].T).astype(BF)
        xT = np.ascontiguousarray(
            xT.reshape(KT, 128, TLOC).transpose(1, 0, 2)
        ).reshape(128, KT * TLOC)
        m = dict(shared)
        m["xT"] = xT
        in_maps.append(m)
    return in_maps


def _run(inputs, trace=False):
    if "nc" not in _CACHE:
        _CACHE["nc"] = _build()
    nc = _CACHE["nc"]
    in_maps = _prep_inputs(inputs)
    res = run_bass_kernel_spmd(nc, in_maps, core_ids=list(range(NCORES)),
                               trace=trace)
    out = np.empty((TOK, D), np.float32)
    for c in range(NCORES):
        r = res.results[c]["outT"].astype(np.float32).reshape(128, OT, TLOC)
        out[c * TLOC:(c + 1) * TLOC, :] = \
            r.transpose(2, 1, 0).reshape(TLOC, D)
    return out.reshape(B, NSEQ, D), res


def kernel(**inputs) -> np.ndarray:
    out, _ = _run(inputs, trace=False)
    return out
